# revision 6
# baseline (speedup 1.0000x reference)
"""Trainium2 Bass kernel for nn_BackProjectionLoss.

Computes mean(|bicubic_downsample(output, 512->128) - input|) where the
downsample is the MATLAB-style antialiased cubic (Keys a=-0.5) separable
filter from the reference, with symmetric padding.

Strategy (pure data parallel, 8 cores, 12 images of 512x512 each):
  The separable downsample is expressed as Y2 = D @ X @ D^T with a dense
  128x512 matrix D (symmetric padding folded into D).  Per image:
    pass 1 : Y1 = D @ X            4 fp32r matmuls, N=512, into PSUM
    transp : Y1^T via PE transpose (4x 128x128 blocks)
    pass 2 : Y2^T = D @ Y1^T       batched 4 images wide (N=512)
    compare: |Y2^T - In^T| summed with a DVE reduce (In transposed on PE)
  Each core emits per-partition partial sums [128, n_groups]; the host
  sums them in float64 and divides by the element count.
"""

import numpy as np
from math import ceil

import concourse.bass as bass
import concourse.mybir as mybir
import concourse.tile as tile
from concourse import bacc
from concourse.bass_utils import run_bass_kernel_spmd
from concourse.masks import make_identity

F32 = mybir.dt.float32
F32R = mybir.dt.float32r

N_CORES = 8
B, C, H, W = 32, 3, 512, 512
h, w = 128, 128
N_IMG = B * C                 # 96 images
IMG_PER_CORE = N_IMG // N_CORES   # 12
GROUP = 4                     # images batched per pass-2 matmul
N_GROUPS = IMG_PER_CORE // GROUP  # 3


def _downsample_matrix():
    """128x512 antialiased-cubic downsample matrix, symmetric pad folded in.

    Mirrors reference._make_kernel(512, 128) + jnp.pad(mode="symmetric") +
    strided valid conv.
    """
    in_len, out_len = H, h
    scale = out_len / in_len
    klen = 4.0 / scale
    x = np.array([1, out_len], dtype=np.float64)
    u = x / scale + 0.5 * (1.0 - 1.0 / scale)
    left = np.floor(u - klen / 2.0)
    p = int(ceil(klen)) + 2
    ind = left[:, None] + np.arange(p) - 1
    indices = ind.astype(np.int64)
    xx = (u[:, None] - indices - 1) * scale
    ax = np.abs(xx)
    ax2 = ax * ax
    ax3 = ax2 * ax
    wgt = (1.5 * ax3 - 2.5 * ax2 + 1.0) * (ax <= 1) + \
          (-0.5 * ax3 + 2.5 * ax2 - 4.0 * ax + 2.0) * ((ax > 1) & (ax <= 2))
    wgt = wgt * scale
    wgt = wgt / wgt.sum(axis=1, keepdims=True)
    keep = np.nonzero(np.any(wgt, axis=0))[0]
    wgt = wgt[:, keep]
    indices = indices[:, keep]
    assert np.all(wgt[0] == wgt[-1])
    pad_l = int(np.where(indices[0] == 0)[0][0])
    taps = wgt[0].astype(np.float32)
    L = taps.shape[0]
    stride = in_len // out_len

    D = np.zeros((out_len, in_len), dtype=np.float64)
    for i in range(out_len):
        for t in range(L):
            s = stride * i + t - pad_l
            if s < 0:
                s = -s - 1
            if s >= in_len:
                s = 2 * in_len - 1 - s
            D[i, s] += float(taps[t])
    return D.astype(np.float32)


def _build_program():
    Dp = _downsample_matrix()                      # [128, 512]
    # dmat[kc, k, i] = D[i, kc*128 + k]  (D^T in 4 chunks of 128 rows)
    dmat = np.ascontiguousarray(
        Dp.T.reshape(4, 128, 128)).astype(np.float32)

    nc = bacc.Bacc("TRN2", target_bir_lowering=False, debug=False,
                   num_devices=N_CORES)
    # float32r: same 4-byte layout as float32; tags the PE fast-path
    # (1 cycle/row at N>=512) and satisfies the walrus fp32r-producer rule.
    x_d = nc.dram_tensor("x", [IMG_PER_CORE, H, W], F32R, kind="ExternalInput")
    in_d = nc.dram_tensor("inp", [IMG_PER_CORE, h, w], F32,
                          kind="ExternalInput")
    out_d = nc.dram_tensor("out", [128, N_GROUPS], F32, kind="ExternalOutput")
    dmat_d = nc.inline_tensor(dmat, "dmat")        # [4, 128, 128]

    with tile.TileContext(nc) as tc:
        with (
            tc.tile_pool(name="const", bufs=1) as const_pool,
            tc.tile_pool(name="xp", bufs=3) as x_pool,
            tc.tile_pool(name="y1s", bufs=2) as y1s_pool,
            tc.tile_pool(name="rhs", bufs=2) as rhs_pool,
            tc.tile_pool(name="inb", bufs=2) as in_pool,
            tc.tile_pool(name="ints", bufs=2) as ints_pool,
            tc.tile_pool(name="diff", bufs=2) as diff_pool,
            tc.tile_pool(name="y1p", bufs=2, space="PSUM") as y1_psum,
            tc.tile_pool(name="y1tp", bufs=2, space="PSUM") as y1t_psum,
            tc.tile_pool(name="y2tp", bufs=2, space="PSUM") as y2t_psum,
            tc.tile_pool(name="intp", bufs=2, space="PSUM") as int_psum,
        ):
            dt_raw = const_pool.tile([128, 4, 128], F32)
            nc.sync.dma_start(out=dt_raw,
                              in_=dmat_d.ap().rearrange("c k i -> k c i"))
            dt_sb = const_pool.tile([128, 4, 128], F32R)
            nc.vector.tensor_copy(out=dt_sb, in_=dt_raw)
            ident = const_pool.tile([128, 128], F32)
            make_identity(nc, ident)
            acc = const_pool.tile([128, N_GROUPS], F32)

            for grp in range(N_GROUPS):
                rhs_sb = rhs_pool.tile([128, 4, GROUP * 128], F32R)
                in_sb = in_pool.tile([128, GROUP, 128], F32)
                nc.sync.dma_start(
                    out=in_sb,
                    in_=in_d[grp * GROUP:(grp + 1) * GROUP].rearrange(
                        "g i j -> i g j"))
                int_ps = int_psum.tile([128, GROUP * 128], F32)

                for ig in range(GROUP):
                    im = grp * GROUP + ig
                    x_sb = x_pool.tile([128, 4, 512], F32R)
                    nc.sync.dma_start(
                        out=x_sb,
                        in_=x_d[im].rearrange("(kc k) c -> k kc c", k=128))
                    y1_ps = y1_psum.tile([128, 512], F32)
                    for kc in range(4):
                        nc.tensor.matmul(
                            y1_ps,
                            dt_sb[:, kc, :],
                            x_sb[:, kc, :],
                            start=(kc == 0), stop=(kc == 3))
                    y1_sb = y1s_pool.tile([128, 512], F32)
                    nc.vector.tensor_copy(out=y1_sb, in_=y1_ps)
                    y1t_ps = y1t_psum.tile([128, 512], F32)
                    for cc in range(4):
                        nc.tensor.transpose(
                            y1t_ps[:, cc * 128:(cc + 1) * 128],
                            y1_sb[:, cc * 128:(cc + 1) * 128],
                            ident)
                    nc.vector.tensor_copy(
                        out=rhs_sb[:, :, ig * 128:(ig + 1) * 128],
                        in_=y1t_ps.rearrange("p (c i) -> p c i", c=4))
                    nc.tensor.transpose(
                        int_ps[:, ig * 128:(ig + 1) * 128],
                        in_sb[:, ig, :],
                        ident)

                int_sb = ints_pool.tile([128, GROUP * 128], F32)
                nc.vector.tensor_copy(out=int_sb, in_=int_ps)
                y2t_ps = y2t_psum.tile([128, GROUP * 128], F32)
                for cc in range(4):
                    nc.tensor.matmul(
                        y2t_ps,
                        dt_sb[:, cc, :],
                        rhs_sb[:, cc, :],
                        start=(cc == 0), stop=(cc == 3))
                diff_sb = diff_pool.tile([128, GROUP * 128], F32)
                nc.vector.tensor_sub(diff_sb, y2t_ps, int_sb)
                nc.vector.tensor_reduce(
                    out=acc[:, grp:grp + 1], in_=diff_sb,
                    axis=mybir.AxisListType.X, op=mybir.AluOpType.add,
                    apply_absolute_value=True)

            nc.sync.dma_start(out=out_d.ap(), in_=acc)

    nc.compile()
    return nc


_PROGRAM = None


def _get_program():
    global _PROGRAM
    if _PROGRAM is None:
        _PROGRAM = _build_program()
    return _PROGRAM


def _shard_inputs(input, output):
    xs = np.ascontiguousarray(
        np.asarray(output, dtype=np.float32).reshape(N_IMG, H, W))
    ins = np.ascontiguousarray(
        np.asarray(input, dtype=np.float32).reshape(N_IMG, h, w))
    in_maps = []
    for c in range(N_CORES):
        sl = slice(c * IMG_PER_CORE, (c + 1) * IMG_PER_CORE)
        in_maps.append({
            "x": np.ascontiguousarray(xs[sl]),
            "inp": np.ascontiguousarray(ins[sl]),
        })
    return in_maps


def _run(input, output, **kwargs):
    nc = _get_program()
    in_maps = _shard_inputs(input, output)
    res = run_bass_kernel_spmd(nc, in_maps, core_ids=list(range(N_CORES)),
                               **kwargs)
    total = 0.0
    for r in res.results:
        total += r["out"].astype(np.float64).sum()
    mean = total / float(N_IMG * h * w)
    return np.float32(mean), res


def kernel(input, output):
    val, _ = _run(input, output)
    return np.asarray(val, dtype=np.float32)
